# revision 5
# baseline (speedup 1.0000x reference)
"""PillarMaxPooling Trainium2 kernel (8 NeuronCores, SPMD).

Strategy
--------
Output (pillar) sharding: core c owns pillars [c*PPC, (c+1)*PPC).
Host-side prep is pure indexing/sharding work: points are routed to the
core that owns their pillar and packed into a fixed 16-slot-per-pillar
layout (pillars with >16 points spill into "virtual pillars" at the
tail of the core's pillar space, combined on-device at the end).

BatchNorm folding: z = x @ (W * inv_std) + shift is expressed as a
single matmul by appending a constant-1 feature carrying `shift`, so
the device computes z directly, max-pools over each pillar's 16 slots,
and applies ReLU.  ReLU commutes with max, and all-zero padding slots
are exact neutral elements because max(relu(a), 0-slot) == relu(max).

Device program per core (identical program on all 8 cores):
  - xs  [128, G*256] fp16 : slot features; pillar q=(g,j) has 16 slots
        split over column-tiles 2g, 2g+1 (8 slots each, r-interleaved
        across the 128-partition contraction dim as 8 blocks of 16
        features).
  - w8  [128, 512]  fp16 : block-diagonal folded weights; matmul of an
        xs column-tile against w8 yields z for 8 slots x 64 channels.
  - per pillar group: 2 matmuls -> f32 PSUM, ACT relu-drains tile 0,
    DVE max-combines tile 1 + a 3-level max tree -> [128, 64] f32.
  - virtual pillars are folded into their real rows via indirect DMA
    gather + DVE max + indirect DMA scatter.
"""

import os
import numpy as np

P_TOTAL = 2_000_000
C_IN = 10
C_OUT = 64
M_PILLARS = 200_000
N_CORES = 8
BN_EPS = 1e-3
K_SLOTS = 16          # slots per pillar (2 column-tiles x 8)
F_PAD = 16            # features padded: 10 real + 1 const + 5 zero
CHUNK_G = 8           # pillar groups per DMA chunk

# stash of the last device-run results (exec_time_ns etc) for test harnesses
LAST_RESULTS = None

_PROGRAM_CACHE = {}


def _build_program(G, VCHUNKS, trace):
    """Build the (core-agnostic) bass program for G pillar groups and
    VCHUNKS virtual-fixup batches of 128 rows each."""
    import concourse.bass as bass
    import concourse.tile as tile
    from concourse import bacc, mybir

    F16 = mybir.dt.float16
    F32 = mybir.dt.float32
    I32 = mybir.dt.int32

    nc = bacc.Bacc(None)
    xs_d = nc.declare_dram_parameter("xs", [128, G * 256], F16, isOutput=False)
    w8_d = nc.declare_dram_parameter("w8", [128, 512], F16, isOutput=False)
    if VCHUNKS:
        vg_d = nc.declare_dram_parameter("vgidx", [128, VCHUNKS], I32, isOutput=False)
        vs_d = nc.declare_dram_parameter("vsidx", [128, VCHUNKS], I32, isOutput=False)
    out_d = nc.declare_dram_parameter("out", [128, G * 64], F32, isOutput=True)
    out_rows = out_d.ap().rearrange("p (g d) -> (p g) d", d=64)

    n_chunks = (G + CHUNK_G - 1) // CHUNK_G

    with tile.TileContext(nc) as tc:
        with (
            tc.tile_pool(name="wp", bufs=1) as wp,
            tc.tile_pool(name="xsp", bufs=3) as xsp,
            tc.tile_pool(name="ps", bufs=4, space="PSUM") as ps,
            tc.tile_pool(name="sp", bufs=3) as sp,
            tc.tile_pool(name="tp", bufs=3) as tp,
            tc.tile_pool(name="stg", bufs=3) as stg,
            tc.tile_pool(name="vx", bufs=1) as vx,
        ):
            w8 = wp.tile([128, 512], F16)
            nc.sync.dma_start(out=w8[:], in_=w8_d[:])

            for ch in range(n_chunks):
                g0 = ch * CHUNK_G
                g1 = min(g0 + CHUNK_G, G)
                ng = g1 - g0
                xs = xsp.tile([128, CHUNK_G * 256], F16, tag="xs")
                nc.sync.dma_start(
                    out=xs[:, : ng * 256], in_=xs_d[:, g0 * 256 : g1 * 256]
                )
                staging = stg.tile([128, CHUNK_G * 64], F32, tag="stg")
                for gl in range(ng):
                    p0 = ps.tile([128, 512], F32, tag="psum")
                    p1 = ps.tile([128, 512], F32, tag="psum")
                    nc.tensor.matmul(
                        p0[:], xs[:, gl * 256 : gl * 256 + 128], w8[:],
                        start=True, stop=True,
                    )
                    nc.tensor.matmul(
                        p1[:], xs[:, gl * 256 + 128 : gl * 256 + 256], w8[:],
                        start=True, stop=True,
                    )
                    s0 = sp.tile([128, 512], F16, tag="s0")
                    nc.scalar.activation(
                        out=s0[:], in_=p0[:],
                        func=mybir.ActivationFunctionType.Relu,
                    )
                    s1 = sp.tile([128, 512], F16, tag="s1")
                    nc.vector.tensor_max(s1[:], p1[:], s0[:])
                    t1 = tp.tile([128, 256], F16, tag="t1")
                    nc.vector.tensor_max(t1[:], s1[:, 0:256], s1[:, 256:512])
                    t2 = tp.tile([128, 128], F16, tag="t2")
                    nc.vector.tensor_max(t2[:], t1[:, 0:128], t1[:, 128:256])
                    nc.vector.tensor_max(
                        staging[:, gl * 64 : (gl + 1) * 64],
                        t2[:, 0:64], t2[:, 64:128],
                    )
                nc.sync.dma_start(
                    out=out_d[:, g0 * 64 : g1 * 64], in_=staging[:, : ng * 64]
                )

            # ---- virtual pillar fixup -------------------------------------
            if VCHUNKS:
                vg = vx.tile([128, VCHUNKS], I32)
                vs = vx.tile([128, VCHUNKS], I32)
                nc.sync.dma_start(out=vg[:], in_=vg_d[:])
                nc.sync.dma_start(out=vs[:], in_=vs_d[:])
                for b in range(VCHUNKS):
                    vrow = sp.tile([128, 64], F32, tag="vrow")
                    trow = sp.tile([128, 64], F32, tag="trow")
                    mrow = sp.tile([128, 64], F32, tag="mrow")
                    nc.gpsimd.indirect_dma_start(
                        out=vrow[:], out_offset=None,
                        in_=out_rows,
                        in_offset=bass.IndirectOffsetOnAxis(
                            ap=vg[:, b : b + 1], axis=0),
                    )
                    nc.gpsimd.indirect_dma_start(
                        out=trow[:], out_offset=None,
                        in_=out_rows,
                        in_offset=bass.IndirectOffsetOnAxis(
                            ap=vs[:, b : b + 1], axis=0),
                    )
                    nc.vector.tensor_max(mrow[:], vrow[:], trow[:])
                    nc.gpsimd.indirect_dma_start(
                        out=out_rows,
                        out_offset=bass.IndirectOffsetOnAxis(
                            ap=vs[:, b : b + 1], axis=0),
                        in_=mrow[:], in_offset=None,
                    )
    nc.finalize()
    return nc


def _ensure_ntff_hook():
    """Install the antenv.axon_hooks shim if the image lacks it, wiring the
    NTFF profile hook straight to libaxon_pjrt.so (trace-only path)."""
    import sys
    import types
    try:
        from antenv.axon_hooks import get_axon_ntff_profile_hook  # noqa: F401
        return
    except ImportError:
        pass
    import antenv
    from trn_agent_boot.trn_boot import _ntff_profile_via_ctypes
    mod = types.ModuleType("antenv.axon_hooks")
    hook = [_ntff_profile_via_ctypes("/opt/axon/libaxon_pjrt.so")]
    mod.get_axon_ntff_profile_hook = lambda: hook[0]
    mod.set_axon_ntff_profile_hook = lambda h: hook.__setitem__(0, h)
    sys.modules["antenv.axon_hooks"] = mod
    antenv.axon_hooks = mod


def kernel(group_features, pillar_set_indices, num_pillars, W, gamma, beta,
           running_mean, running_var):
    global LAST_RESULTS
    from concourse.bass_utils import run_bass_kernel_spmd

    x = np.ascontiguousarray(np.asarray(group_features, dtype=np.float32))
    idx = np.asarray(pillar_set_indices).astype(np.int64)
    M = int(num_pillars)
    P = x.shape[0]
    ppc = (M + N_CORES - 1) // N_CORES  # pillars per core

    # ---- fold BN into the weights (tiny parameter math) -------------------
    inv_std = np.asarray(gamma, np.float32) / np.sqrt(
        np.asarray(running_var, np.float32) + BN_EPS)
    Wt = np.zeros((F_PAD, C_OUT), np.float32)
    Wt[:C_IN] = np.asarray(W, np.float32) * inv_std[None, :]
    Wt[C_IN] = (np.asarray(beta, np.float32)
                - np.asarray(running_mean, np.float32) * inv_std)
    w8 = np.zeros((8, F_PAD, 512), np.float16)
    for r in range(8):
        w8[r, :, r * 64 : (r + 1) * 64] = Wt
    w8 = w8.reshape(128, 512)

    # ---- route points to pillar-owning cores (sharding) -------------------
    order = np.argsort(idx, kind="stable")
    idx_s = idx[order]
    x_s = x[order]
    counts = np.bincount(idx_s, minlength=M)
    starts = np.zeros(M + 1, np.int64)
    np.cumsum(counts, out=starts[1:])
    rank = np.arange(P, dtype=np.int64) - starts[idx_s]

    cores = []
    G_need, NV_need = 0, 0
    for c in range(N_CORES):
        plo = c * ppc
        phi = min(plo + ppc, M)
        sl = slice(starts[plo], starts[phi])
        pid = idx_s[sl] - plo
        cnt = counts[plo:phi]
        n_virt = np.maximum((cnt.astype(np.int64) - 1) // K_SLOTS, 0)
        nv = int(n_virt.sum())
        cores.append((plo, phi, pid, x_s[sl], rank[sl], n_virt, nv))
        G_need = max(G_need, (ppc + nv + 127) // 128)
        vb = 0  # fixup batches: each chain level padded to full batches
        lv = 0
        while True:
            cl = int((n_virt > lv).sum())
            if cl == 0:
                break
            vb += (cl + 127) // 128
            lv += 1
        NV_need = max(NV_need, vb)
    G = G_need + 1  # ≥1 spare dummy row (trash target for padded fixups)
    VCHUNKS = NV_need
    NP = G * 128

    # ---- per-core packing -------------------------------------------------
    in_maps = []
    for c in range(N_CORES):
        plo, phi, pid, xs_pts, rk, n_virt, nv = cores[c]
        npil = phi - plo
        virt_base = np.zeros(npil, np.int64)
        np.cumsum(n_virt[:-1], out=virt_base[1:])
        chain = rk // K_SLOTS
        eff = np.where(chain == 0, pid, npil + virt_base[pid] + chain - 1)
        slot = rk % K_SLOTS

        slots_np = np.zeros((NP, K_SLOTS, F_PAD), np.float16)
        slots_np[eff, slot, :C_IN] = xs_pts.astype(np.float16)
        slots_np[eff, slot, C_IN] = 1.0
        # [q=(g,j), s=(d,r), f] -> xs[16r+f, (2g+d)*128 + j]
        A = slots_np.reshape(G, 128, 2, 8, F_PAD)
        xs_dev = np.ascontiguousarray(
            A.transpose(3, 4, 0, 2, 1).reshape(128, G * 256))

        im = {"xs": xs_dev, "w8": w8}
        if VCHUNKS:
            # virtual row v (device pillar npil+v) folds into real pillar.
            # A pillar with chained virtuals (>32 points) appears multiple
            # times as a target; those must go to *different* sequential
            # batches, so order fixups by chain level and pad each level
            # to a full batch of 128.
            tgt = np.repeat(np.arange(npil, dtype=np.int64), n_virt)
            vq = npil + np.arange(nv, dtype=np.int64)   # virtual pillar ids
            lvl = np.concatenate(
                [np.arange(k, dtype=np.int64) for k in n_virt if k]
            ) if nv else np.zeros(0, np.int64)
            trash = NP - 1                               # spare dummy row
            row = lambda q: (q % 128) * G + q // 128     # device row id
            gq_l, sq_l = [], []
            for lv in range(int(lvl.max()) + 1 if nv else 0):
                m = lvl == lv
                g_lvl, s_lvl = row(vq[m]), row(tgt[m])
                pad = (-len(g_lvl)) % 128
                gq_l.append(np.pad(g_lvl, (0, pad), constant_values=row(trash)))
                sq_l.append(np.pad(s_lvl, (0, pad), constant_values=row(trash)))
            gq = np.concatenate(gq_l) if gq_l else np.zeros(0, np.int64)
            sq = np.concatenate(sq_l) if sq_l else np.zeros(0, np.int64)
            pad = VCHUNKS * 128 - len(gq)
            assert pad >= 0, "VCHUNKS underestimated"
            gq = np.pad(gq, (0, pad), constant_values=row(trash))
            sq = np.pad(sq, (0, pad), constant_values=row(trash))
            im["vgidx"] = np.ascontiguousarray(
                gq.reshape(VCHUNKS, 128).T.astype(np.int32))
            im["vsidx"] = np.ascontiguousarray(
                sq.reshape(VCHUNKS, 128).T.astype(np.int32))
        in_maps.append(im)

    # ---- build + run ------------------------------------------------------
    key = (G, VCHUNKS)
    if key not in _PROGRAM_CACHE:
        _PROGRAM_CACHE[key] = _build_program(G, VCHUNKS, trace=False)
    nc = _PROGRAM_CACHE[key]

    trace = bool(int(os.environ.get("PILLAR_TRACE", "0")))
    if trace:
        _ensure_ntff_hook()
    res = run_bass_kernel_spmd(
        nc, in_maps, core_ids=list(range(N_CORES)), trace=trace,
        trace_cores=list(range(N_CORES)) if trace else None,
    )
    LAST_RESULTS = res

    # ---- unshard ----------------------------------------------------------
    out_full = np.zeros((M, C_OUT), np.float32)
    for c in range(N_CORES):
        plo, phi, *_ = cores[c]
        dev = res.results[c]["out"].reshape(128, G, C_OUT)
        rows = dev.transpose(1, 0, 2).reshape(NP, C_OUT)
        out_full[plo:phi] = rows[: phi - plo]
    return out_full


# revision 9
# speedup vs baseline: 1.3636x; 1.3636x over previous
"""PillarMaxPooling Trainium2 kernel (8 NeuronCores, SPMD).

Strategy
--------
Output (pillar) sharding: core c owns pillars [c*PPC, (c+1)*PPC).
Host-side prep is pure indexing/sharding work: points are routed to the
core that owns their pillar and packed into fixed per-pillar slot rows.
Pillars are stratified by point count into depth classes {1, 2, 4}
(8/16/32 slots); pillars with >32 points spill the excess into
"virtual pillar" entities combined on-device at the end.

BatchNorm folding: z = x @ (W * inv_std) + shift is one matmul via an
appended constant-1 feature carrying `shift`.  ReLU commutes with max,
and all-zero padding slots are exact neutral elements because
max(relu(a), 0) == relu(max(a, anything <= relu)).

Device program per core (identical program on all 8 cores):
  - xs  [128, NT*128] fp16 : slot features; entity (g,j) of depth d owns
        column-tiles [base(g) .. base(g)+d) at column j; each tile column
        packs 8 slots x 16 features down the 128-partition contraction.
  - w8  [128, 512] fp16 : block-diagonal folded weights; one matmul of an
        xs tile against w8 yields z for 8 slots x 64 channels (f32 PSUM).
  - per group: ACT relu-drains even PSUM tiles, DVE max-combines odd
        tiles, GPSIMD + DVE run the max tree -> [128, 64] f32 output rows.
"""

import os
import numpy as np

C_IN = 10
C_OUT = 64
N_CORES = 8
BN_EPS = 1e-3
F_PAD = 16            # features padded: 10 real + 1 const + 5 zero
MAX_SLOTS = 32        # slots per entity cap (depth class 4)
CHUNK_TILES = 32      # xs column-tiles per DMA chunk
D2_ACT_BOTH_MOD = 5   # every Nth depth-2 group: ACT drains both tiles
D1_DVE_MOD = 0        # 0: ACT drains all depth-1 groups

LAST_RESULTS = None
_PROGRAM_CACHE = {}


def _ensure_ntff_hook():
    """Install the antenv.axon_hooks shim if the image lacks it, wiring the
    NTFF profile hook straight to libaxon_pjrt.so (trace-only path)."""
    import sys
    import types
    try:
        from antenv.axon_hooks import get_axon_ntff_profile_hook  # noqa: F401
        return
    except ImportError:
        pass
    import antenv
    from trn_agent_boot.trn_boot import _ntff_profile_via_ctypes
    mod = types.ModuleType("antenv.axon_hooks")
    hook = [_ntff_profile_via_ctypes("/opt/axon/libaxon_pjrt.so")]
    mod.get_axon_ntff_profile_hook = lambda: hook[0]
    mod.set_axon_ntff_profile_hook = lambda h: hook.__setitem__(0, h)
    sys.modules["antenv.axon_hooks"] = mod
    antenv.axon_hooks = mod


def _build_program(G4, G2, G1, VCHUNKS):
    import concourse.bass as bass
    import concourse.tile as tile
    from concourse import bacc, mybir

    F16 = mybir.dt.float16
    F32 = mybir.dt.float32
    I32 = mybir.dt.int32
    MAX = mybir.AluOpType.max
    Gtot = G4 + G2 + G1
    NT = 4 * G4 + 2 * G2 + G1

    nc = bacc.Bacc(None)
    xs_d = nc.declare_dram_parameter("xs", [128, NT * 128], F16, isOutput=False)
    w8_d = nc.declare_dram_parameter("w8", [128, 512], F16, isOutput=False)
    if VCHUNKS:
        vg_d = nc.declare_dram_parameter("vgidx", [128, VCHUNKS], I32, isOutput=False)
        vs_d = nc.declare_dram_parameter("vsidx", [128, VCHUNKS], I32, isOutput=False)
    out_d = nc.declare_dram_parameter("out", [128, Gtot * 64], F32, isOutput=True)
    out_rows = out_d.ap().rearrange("p (g d) -> (p g) d", d=64)

    # (depth, n_groups, tile_base, group_base, act_drain_mod) per class;
    # act_drain_mod: for depth-1 groups, which take the ACT drain path.
    classes = [(4, G4, 0, 0), (2, G2, 4 * G4, G4), (1, G1, 4 * G4 + 2 * G2, G4 + G2)]

    with tile.TileContext(nc) as tc:
        with (
            tc.tile_pool(name="wp", bufs=1) as wp,
            tc.tile_pool(name="xsp", bufs=3) as xsp,
            tc.tile_pool(name="ps", bufs=8, space="PSUM") as ps,
            tc.tile_pool(name="sp", bufs=4) as sp,
            tc.tile_pool(name="tp", bufs=4) as tp,
            tc.tile_pool(name="stg", bufs=3) as stg,
            tc.tile_pool(name="vx", bufs=1) as vx,
        ):
            w8 = wp.tile([128, 512], F16)
            nc.sync.dma_start(out=w8[:], in_=w8_d[:])

            for depth, ngroups, tbase, gbase in classes:
                if ngroups == 0:
                    continue
                gp_chunk = max(CHUNK_TILES // depth, 1)
                for c0 in range(0, ngroups, gp_chunk):
                    c1 = min(c0 + gp_chunk, ngroups)
                    ntile = (c1 - c0) * depth
                    t0 = tbase + c0 * depth
                    xs = xsp.tile([128, CHUNK_TILES * 128], F16, tag="xs")
                    nc.sync.dma_start(
                        out=xs[:, : ntile * 128],
                        in_=xs_d[:, t0 * 128 : (t0 + ntile) * 128],
                    )
                    staging = stg.tile([128, gp_chunk * 64], F16, tag="stg")
                    staging32 = stg.tile([128, gp_chunk * 64], F32, tag="stg32")
                    for gl in range(c1 - c0):
                        col = gl * depth * 128
                        pt = [ps.tile([128, 512], F32, tag="psum",
                                      name=f"pt{t}")
                              for t in range(depth)]
                        for t in range(depth):
                            nc.tensor.matmul(
                                pt[t][:],
                                xs[:, col + t * 128 : col + (t + 1) * 128],
                                w8[:], start=True, stop=True,
                            )
                        if depth == 4:
                            a0 = sp.tile([128, 512], F16, tag="a0")
                            a2 = sp.tile([128, 512], F16, tag="a2")
                            nc.scalar.activation(
                                out=a0[:], in_=pt[0][:],
                                func=mybir.ActivationFunctionType.Relu)
                            nc.scalar.activation(
                                out=a2[:], in_=pt[2][:],
                                func=mybir.ActivationFunctionType.Relu)
                            s1a = sp.tile([128, 512], F16, tag="s1a")
                            s1b = sp.tile([128, 512], F16, tag="s1b")
                            nc.vector.tensor_max(s1a[:], pt[1][:], a0[:])
                            nc.vector.tensor_max(s1b[:], pt[3][:], a2[:])
                            td = tp.tile([128, 512], F16, tag="td")
                            nc.vector.tensor_max(td[:], s1a[:], s1b[:])
                            t1 = tp.tile([128, 256], F16, tag="t1")
                            nc.vector.tensor_max(t1[:], td[:, 0:256], td[:, 256:512])
                        elif depth == 2:
                            a0 = sp.tile([128, 512], F16, tag="a0")
                            nc.scalar.activation(
                                out=a0[:], in_=pt[0][:],
                                func=mybir.ActivationFunctionType.Relu)
                            s1 = sp.tile([128, 512], F16, tag="s1a")
                            if D2_ACT_BOTH_MOD and (c0 + gl) % D2_ACT_BOTH_MOD == 0:
                                a1 = sp.tile([128, 512], F16, tag="a1")
                                nc.scalar.activation(
                                    out=a1[:], in_=pt[1][:],
                                    func=mybir.ActivationFunctionType.Relu)
                                nc.vector.tensor_max(s1[:], a1[:], a0[:])
                            else:
                                nc.vector.tensor_max(s1[:], pt[1][:], a0[:])
                            t1 = tp.tile([128, 256], F16, tag="t1")
                            nc.vector.tensor_max(t1[:], s1[:, 0:256], s1[:, 256:512])
                        else:  # depth 1
                            s1 = sp.tile([128, 512], F16, tag="s1a")
                            if D1_DVE_MOD and (c0 + gl) % D1_DVE_MOD == 0:
                                nc.vector.tensor_copy(s1[:], pt[0][:])
                            else:
                                nc.scalar.activation(
                                    out=s1[:], in_=pt[0][:],
                                    func=mybir.ActivationFunctionType.Relu)
                            t1 = tp.tile([128, 256], F16, tag="t1")
                            nc.vector.tensor_max(t1[:], s1[:, 0:256], s1[:, 256:512])
                        t2 = tp.tile([128, 128], F16, tag="t2")
                        nc.vector.tensor_max(t2[:], t1[:, 0:128], t1[:, 128:256])
                        # out = max(max(t2a, 0), t2b): final combine + ReLU
                        nc.vector.scalar_tensor_tensor(
                            out=staging[:, gl * 64 : (gl + 1) * 64],
                            in0=t2[:, 0:64], scalar=0.0, in1=t2[:, 64:128],
                            op0=MAX, op1=MAX,
                        )
                    nc.vector.tensor_copy(
                        staging32[:, : (c1 - c0) * 64], staging[:, : (c1 - c0) * 64])
                    nc.sync.dma_start(
                        out=out_d[:, (gbase + c0) * 64 : (gbase + c1) * 64],
                        in_=staging32[:, : (c1 - c0) * 64],
                    )

            if VCHUNKS:
                vg = vx.tile([128, VCHUNKS], I32)
                vs = vx.tile([128, VCHUNKS], I32)
                nc.sync.dma_start(out=vg[:], in_=vg_d[:])
                nc.sync.dma_start(out=vs[:], in_=vs_d[:])
                for b in range(VCHUNKS):
                    vrow = sp.tile([128, 64], F32, tag="vrow")
                    trow = sp.tile([128, 64], F32, tag="trow")
                    mrow = sp.tile([128, 64], F32, tag="mrow")
                    nc.gpsimd.indirect_dma_start(
                        out=vrow[:], out_offset=None,
                        in_=out_rows,
                        in_offset=bass.IndirectOffsetOnAxis(
                            ap=vg[:, b : b + 1], axis=0),
                    )
                    nc.gpsimd.indirect_dma_start(
                        out=trow[:], out_offset=None,
                        in_=out_rows,
                        in_offset=bass.IndirectOffsetOnAxis(
                            ap=vs[:, b : b + 1], axis=0),
                    )
                    nc.vector.tensor_max(mrow[:], vrow[:], trow[:])
                    nc.gpsimd.indirect_dma_start(
                        out=out_rows,
                        out_offset=bass.IndirectOffsetOnAxis(
                            ap=vs[:, b : b + 1], axis=0),
                        in_=mrow[:], in_offset=None,
                    )
    nc.finalize()
    return nc


def _depth_of(load):
    d = np.ones_like(load)
    d[load > 8] = 2
    d[load > 16] = 4
    return d


def kernel(group_features, pillar_set_indices, num_pillars, W, gamma, beta,
           running_mean, running_var):
    global LAST_RESULTS
    from concourse.bass_utils import run_bass_kernel_spmd

    x = np.ascontiguousarray(np.asarray(group_features, dtype=np.float32))
    idx = np.asarray(pillar_set_indices).astype(np.int64)
    M = int(num_pillars)
    P = x.shape[0]
    ppc = (M + N_CORES - 1) // N_CORES

    # ---- fold BN into the weights -----------------------------------------
    inv_std = np.asarray(gamma, np.float32) / np.sqrt(
        np.asarray(running_var, np.float32) + BN_EPS)
    Wt = np.zeros((F_PAD, C_OUT), np.float32)
    Wt[:C_IN] = np.asarray(W, np.float32) * inv_std[None, :]
    Wt[C_IN] = (np.asarray(beta, np.float32)
                - np.asarray(running_mean, np.float32) * inv_std)
    w8 = np.zeros((8, F_PAD, 512), np.float16)
    for r in range(8):
        w8[r, :, r * 64 : (r + 1) * 64] = Wt
    w8 = w8.reshape(128, 512)

    # ---- route points to pillar-owning cores ------------------------------
    order = np.argsort(idx, kind="stable")
    idx_s = idx[order]
    x_s = x[order]
    counts = np.bincount(idx_s, minlength=M)
    starts = np.zeros(M + 1, np.int64)
    np.cumsum(counts, out=starts[1:])
    rank = np.arange(P, dtype=np.int64) - starts[idx_s]

    # ---- per-core entity construction (class sizes first) -----------------
    percore = []
    N4 = N2 = N1 = NVB = 0
    for c in range(N_CORES):
        plo = c * ppc
        phi = min(plo + ppc, M)
        npil = phi - plo
        sl = slice(starts[plo], starts[phi])
        cnt = counts[plo:phi].astype(np.int64)
        # entities: chunk 0 of each pillar + overflow chunks (virtual)
        n_chunks = np.maximum((cnt + MAX_SLOTS - 1) // MAX_SLOTS, 1)
        nv = int((n_chunks - 1).sum())
        load_main = np.minimum(cnt, MAX_SLOTS)
        # virtual entity loads: chunks 1.. of overflowing pillars
        vp = np.nonzero(n_chunks > 1)[0]
        vload, vtgt, vlvl = [], [], []
        for p in vp:
            rem = cnt[p] - MAX_SLOTS
            lv = 0
            while rem > 0:
                vload.append(min(rem, MAX_SLOTS))
                vtgt.append(p)
                vlvl.append(lv)
                rem -= MAX_SLOTS
                lv += 1
        vload = np.array(vload, np.int64)
        load = np.concatenate([load_main, vload])
        depth = _depth_of(load)
        n4 = int((depth == 4).sum()); n2 = int((depth == 2).sum())
        n1 = int((depth == 1).sum())
        percore.append((plo, phi, sl, cnt, load, depth, vtgt, vlvl, nv))
        N4 = max(N4, n4); N2 = max(N2, n2); N1 = max(N1, n1)
        # fixup batches (each chain level padded to 128)
        if nv:
            lvl_arr = np.array(vlvl, np.int64)
            vb = sum((int((lvl_arr == lv).sum()) + 127) // 128
                     for lv in range(int(lvl_arr.max()) + 1))
            NVB = max(NVB, vb)
    G4 = (N4 + 127) // 128
    G2 = (N2 + 127) // 128
    G1 = (N1 + 128) // 128  # +1 spare slot for the trash row
    Gtot = G4 + G2 + G1
    NT = 4 * G4 + 2 * G2 + G1
    VCHUNKS = NVB

    # ---- per-core packing -------------------------------------------------
    in_maps = []
    unshard = []
    for c in range(N_CORES):
        plo, phi, sl, cnt, load, depth, vtgt, vlvl, nv = percore[c]
        npil = phi - plo
        ne = npil + nv
        # order entities: class 4, then 2, then 1 (stable)
        pos = np.zeros(ne, np.int64)
        i4 = np.nonzero(depth == 4)[0]
        i2 = np.nonzero(depth == 2)[0]
        i1 = np.nonzero(depth == 1)[0]
        pos[i4] = np.arange(len(i4))
        pos[i2] = G4 * 128 + np.arange(len(i2))
        pos[i1] = (G4 + G2) * 128 + np.arange(len(i1))
        # entity -> (tile base, j); groups are blocks of 128 positions
        g = pos // 128
        j = pos % 128
        dep_of_pos = np.where(g < G4, 4, np.where(g < G4 + G2, 2, 1))
        tbase = np.where(
            g < G4, g * 4,
            np.where(g < G4 + G2, 4 * G4 + (g - G4) * 2,
                     4 * G4 + 2 * G2 + (g - G4 - G2)))
        assert (dep_of_pos >= depth).all()

        # points -> (entity, slot)
        pid = idx_s[sl] - plo
        rk = rank[sl]
        chunk = rk // MAX_SLOTS
        kk = rk % MAX_SLOTS
        # virtual entity index for (pillar, chunk>=1)
        max_chain = (max(vlvl) + 1) if nv else 1
        virt_index = np.full((npil, max_chain), -1, np.int64)
        for v, (p, lv) in enumerate(zip(vtgt, vlvl)):
            virt_index[p, lv] = npil + v
        ent = np.where(chunk == 0, pid,
                       virt_index[pid, np.minimum(chunk - 1, max_chain - 1)])
        assert (ent >= 0).all()
        col = (tbase[ent] + kk // 8) * 128 + j[ent]
        row16 = kk % 8

        xs_dev = np.zeros((8, F_PAD, NT * 128), np.float16)
        xs_dev[row16, :C_IN, col] = x_s[sl].astype(np.float16)
        xs_dev[row16, C_IN, col] = 1.0
        xs_dev = xs_dev.reshape(128, NT * 128)

        im = {"xs": xs_dev, "w8": w8}
        if VCHUNKS:
            # device out row of entity q: j*Gtot + g; trash = first unused
            # depth-1 position (G1 reserves at least one spare)
            erow = j * Gtot + g
            trash = (len(i1) % 128) * Gtot + (G4 + G2 + len(i1) // 128)
            # order fixups by chain level, each level padded to 128
            gq_l, sq_l = [], []
            lvl_arr = np.array(vlvl, np.int64)
            for lv in range(int(lvl_arr.max()) + 1 if nv else 0):
                m = np.nonzero(lvl_arr == lv)[0]
                gl_ = erow[npil + m]
                sl_ = erow[np.array(vtgt, np.int64)[m]]
                pad = (-len(gl_)) % 128
                gq_l.append(np.pad(gl_, (0, pad), constant_values=trash))
                sq_l.append(np.pad(sl_, (0, pad), constant_values=trash))
            gq = (np.concatenate(gq_l) if gq_l else np.zeros(0, np.int64))
            sq = (np.concatenate(sq_l) if sq_l else np.zeros(0, np.int64))
            pad = VCHUNKS * 128 - len(gq)
            assert pad >= 0
            gq = np.pad(gq, (0, pad), constant_values=trash)
            sq = np.pad(sq, (0, pad), constant_values=trash)
            im["vgidx"] = np.ascontiguousarray(
                gq.reshape(VCHUNKS, 128).T.astype(np.int32))
            im["vsidx"] = np.ascontiguousarray(
                sq.reshape(VCHUNKS, 128).T.astype(np.int32))
        in_maps.append(im)
        unshard.append((plo, phi, g[:npil].copy(), j[:npil].copy()))

    # ---- build + run ------------------------------------------------------
    key = (G4, G2, G1, VCHUNKS)
    if key not in _PROGRAM_CACHE:
        _PROGRAM_CACHE[key] = _build_program(G4, G2, G1, VCHUNKS)
    nc = _PROGRAM_CACHE[key]

    trace = bool(int(os.environ.get("PILLAR_TRACE", "0")))
    if trace:
        _ensure_ntff_hook()
    res = run_bass_kernel_spmd(
        nc, in_maps, core_ids=list(range(N_CORES)), trace=trace,
        trace_cores=list(range(N_CORES)) if trace else None,
    )
    LAST_RESULTS = res

    # ---- unshard ----------------------------------------------------------
    out_full = np.zeros((M, C_OUT), np.float32)
    for c in range(N_CORES):
        plo, phi, gg, jj = unshard[c]
        dev = res.results[c]["out"].reshape(128, Gtot, C_OUT)
        out_full[plo:phi] = dev[jj, gg, :]
    return out_full
